# revision 41
# baseline (speedup 1.0000x reference)
"""Banded causal self-attention (band width 64) on 8 trn2 NeuronCores.

Sequence-parallel sharding: core c handles batch c//4, query block c%4
(512 queries of T=2048), recomputing a 64-token k/v halo locally so no
collectives are needed. The host casts inputs to bf16 and transposes x
per core; the device kernel fuses qkv-projection -> banded attention ->
output projection.

Device layouts (per core):
  xt/xt8  [128, 8*576]  x chunk transposed, host-packed in SBUF tile
                        order (bf16 for v, fp8e4m3 for q/k)
  qk^T    [2048, 640]   q/k feature-major; key cols 576:640 zero-padded
                        so every key chunk is a full 128 partitions
  v       [640, 1024]   token-major (rows 576:640 zeroed)
  y^T     [1024, 512]   attention output feature-major
  out     [512, 1024]   tokens x C

The q/k projection runs in fp8e4m3 with DoubleRow perf mode (weights
pre-scaled x64; the exp rescales scores by 0.125/64^2) - q/k errors
are damped by the score scale + softmax, unlike v/proj which stay
bf16. Attention is computed transposed (S^T[key, query]) with all 5
key chunks of a (head, s) pair batched into one [128, 768] PSUM strip,
so a single Exp activation covers the strip; the mask multiplies run
out-of-place, split across DVE (s=0) and GpSimd (s=1). Rowsums come
from voner-stationary matmuls replicated over 64 partitions; their
reciprocal uses the fast approx DVE op. AV/rowsum accumulation uses
per-region start flags (no zero-priming matmuls), and the whole
attention loop is software-pipelined by one head pair. All heavy DMAs
ride the in-order sync queue, host-packed for multi-KB contiguous
DRAM runs and batched so one dma_start issue (~590ns of queue time)
covers ~2.4us of PE work. Softmax skips max-subtraction (scores are
O(1)).
"""

import numpy as np
import ml_dtypes

import concourse.mybir as mybir
import concourse.tile as tile
from concourse import bacc
from concourse import bass_utils

B, T, C, H, D = 2, 2048, 1024, 16, 64
W = 64            # band width: key j visible to query i iff i-64 <= j <= i
N_CORES = 8
QL = 512          # queries per core
HT = QL + W       # tokens incl. halo
HTP = QL + 2 * W  # key columns incl. zero pad (640)
P = 128
KC = C // P       # contraction chunks
NFT = 2 * C // P  # q|k feature slabs
NKC = 5           # key chunks (5 x 128, last half zero-padded)
SW = 768          # batched score-strip width (sum of chunk windows)

QSC = 64.0        # fp8 pre-scale on Wqk (q/k come out x64; exp rescales)

bf16 = mybir.dt.bfloat16
f32 = mybir.dt.float32
Act = mybir.ActivationFunctionType

_CACHE = {}

# per key chunk: column offset in the score strip, query window [cs, ce)
CHUNK_OFF = [0, 128, 320, 512, 704]
CHUNK_CS = [0, 64, 192, 320, 448]
CHUNK_CE = [128, 256, 384, 512, 512]
# AV/rowsum accumulation pieces per chunk: (qlo, qhi, first_writer)
SPLITS = [
    [(0, 128, True)],
    [(64, 128, False), (128, 256, True)],
    [(192, 256, False), (256, 384, True)],
    [(320, 384, False), (384, 512, True)],
    [(448, 512, False)],
]


def _emit(tc, xt, xt8, wqk8, wv, wp, bqk, bvr, bvr0, bpr, maskF, voner, out):
    nc = tc.nc
    with (
        tc.tile_pool(name="const", bufs=1) as const,
        tc.tile_pool(name="wqkp", bufs=3) as wqkp,
        tc.tile_pool(name="attn", bufs=4) as at,
        tc.tile_pool(name="rrp", bufs=2) as rrp,
        tc.tile_pool(name="ot", bufs=3) as ot,
        tc.tile_pool(name="psM", bufs=2, space="PSUM") as psM,
        tc.tile_pool(name="psY", bufs=2, space="PSUM") as psY,
        tc.tile_pool(name="psR", bufs=2, space="PSUM") as psR,
    ):
        # ---- persistent tiles; critical-path DMAs first ----
        # xt/wqk arrive host-packed in SBUF tile order so every DMA runs
        # multi-KB contiguous DRAM reads (256B runs measured ~1/3 peak).
        # All heavy transfers go on the in-order sync queue so the tile
        # scheduler cannot hoist them ahead of the hot wqk slabs; the
        # scalar HWDGE queue only carries the small tiles.
        xt8_sb = const.tile([P, KC, HT], mybir.dt.float8e4)
        xt8_r = xt8.rearrange("p (kc t) -> p kc t", t=HT)
        nc.sync.dma_start(xt8_sb[:, : KC // 2], xt8_r[:, : KC // 2])
        nc.sync.dma_start(xt8_sb[:, KC // 2 :], xt8_r[:, KC // 2 :])
        bqk_sb = const.tile([P, NFT], f32)
        nc.scalar.dma_start(bqk_sb[:], bqk.rearrange("(ft p) -> p ft", p=P))

        xt_sb = const.tile([P, KC, HT], bf16)
        xt_r = xt.rearrange("p (kc t) -> p kc t", t=HT)
        wv_sb = const.tile([P, KC, C], bf16)
        wv_r = wv.rearrange("(kc p) n -> p kc n", p=P)
        wp_sb = const.tile([P, KC, C], bf16)
        wp_r = wp.rearrange("(kc p) n -> p kc n", p=P)
        maskF_sb = const.tile([P, SW], bf16)
        bvr_sb = const.tile([P, C], bf16)
        bvr0_sb = const.tile([P, C], bf16)
        bpr_sb = const.tile([P, C], bf16)
        voner_sb = const.tile([P, NKC, D], bf16)
        nc.scalar.dma_start(bvr_sb[:], bvr[:])
        nc.scalar.dma_start(bvr0_sb[:], bvr0[:])
        nc.scalar.dma_start(maskF_sb[:], maskF[:])
        nc.scalar.dma_start(voner_sb[:], voner.rearrange("(c p) e -> p c e", p=P))
        nc.scalar.dma_start(bpr_sb[:], bpr[:])

        ones_row = const.tile([1, P], bf16)
        nc.gpsimd.memset(ones_row[:], 1.0)
        qkT_sb = const.tile([P, NFT, HTP], bf16)
        nc.gpsimd.memset(qkT_sb[:, :, HT:HTP], 0.0)  # zero key pad columns
        v_sb = const.tile([P, NKC, C], bf16)
        nc.gpsimd.memset(v_sb[W:P, NKC - 1], 0.0)    # zero fake-token rows
        yT_sb = const.tile([P, KC, QL], bf16)

        # ---- phase 1a: qk^T = Wqk8^T @ x8^T (fp8 DoubleRow, feature-major) --
        # slab DMAs are batched in pairs: one ~590ns dma_start issue per
        # ~2.4us of PE work, so the sync queue's issue rate can keep ahead
        DR = mybir.MatmulPerfMode.DoubleRow
        for fp in range(NFT // 2):
            wt = wqkp.tile([P, 2, KC, P], mybir.dt.float8e4, tag="wqk")
            nc.sync.dma_start(
                wt[:],
                wqk8[fp].rearrange("p (two kc f) -> p two kc f", two=2, f=P),
            )
            # x (bf16, for v) and Wv ride along behind the hot wqk pairs,
            # spread thin enough that no pair transfer is ever delayed
            if 1 <= fp <= 4:
                qs = slice(2 * (fp - 1), 2 * fp)
                nc.sync.dma_start(xt_sb[:, qs], xt_r[:, qs])
            if 4 <= fp < 8:
                c2 = 2 * (fp - 4)
                nc.sync.dma_start(wv_sb[:, c2:c2 + 2], wv_r[:, c2:c2 + 2])
            for half in (0, 1):
                ft = 2 * fp + half
                # q is only needed for owned tokens (64:576); k needs all
                # 576, written as two segment regions of ONE psum tile so
                # each slab costs a single slot + a single activation
                segs = ((W, QL),) if ft < KC else ((QL, W), (0, QL))
                psf = psM.tile([P, 2 * QL], f32, tag="ps", name="ps1a")
                for t0, tsz in segs:
                    ps = psf[:, t0:t0 + tsz] if ft >= KC else psf[:, :tsz]
                    for kp in range(KC // 2):
                        nc.tensor.matmul(
                            ps, wt[:, half, 2 * kp:2 * kp + 2],
                            xt8_sb[:, 2 * kp:2 * kp + 2, t0:t0 + tsz],
                            start=(kp == 0), stop=(kp == KC // 2 - 1),
                            perf_mode=DR,
                        )
                if ft < KC:
                    nc.scalar.activation(
                        qkT_sb[:, ft, W:W + QL], psf[:, :QL], Act.Identity,
                        bias=bqk_sb[:, ft:ft + 1],
                    )
                else:
                    nc.scalar.activation(
                        qkT_sb[:, ft, :HT], psf[:, :HT], Act.Identity,
                        bias=bqk_sb[:, ft:ft + 1],
                    )

        for c2 in range(0, KC, 2):  # wp behind wv, still ordered on sync
            nc.sync.dma_start(wp_sb[:, c2:c2 + 2], wp_r[:, c2:c2 + 2])

        # ---- phase 1b: v = x @ Wv (token-major) ----
        for tt in range(NKC):
            tsz = P if tt < NKC - 1 else W
            bsel = bvr0_sb if tt == 0 else bvr_sb
            for n0 in (0, QL):
                psf = psM.tile([P, 2 * QL], f32, tag="ps", name="ps1b")
                ps = psf[:tsz, :QL]
                for kc in range(KC):
                    nc.tensor.matmul(
                        ps, xt_sb[:, kc, tt * P:tt * P + tsz],
                        wv_sb[:, kc, n0:n0 + QL],
                        start=(kc == 0), stop=(kc == KC - 1),
                    )
                nc.vector.tensor_add(
                    v_sb[:tsz, tt, n0:n0 + QL], ps, bsel[:tsz, n0:n0 + QL],
                )

        # ---- phase 2: banded attention, transposed-S form ----
        # software-pipelined by one head pair: scores/exp/mask of hp are
        # emitted before the AV/rowsum/normalize of hp-1, so the PE streams
        # the next pair's scores while ACT/GpSimd work on the current one.
        prev = None
        for hp in range(H // 2 + 1):
            cur = None
            if hp < H // 2:
                yA = psY.tile([P, QL], f32, tag="yA")
                rs = psR.tile([P, QL], f32, tag="rs")
                Pes = []
                for s in (0, 1):
                    r0 = D * s
                    psS = psM.tile([P, 2 * QL], f32, tag="ps", name="psS")
                    for c in range(NKC):
                        off, cs, ce = CHUNK_OFF[c], CHUNK_CS[c], CHUNK_CE[c]
                        nc.tensor.matmul(
                            psS[:, off:off + ce - cs],
                            qkT_sb[r0:r0 + D, KC + hp, c * P:(c + 1) * P],
                            qkT_sb[r0:r0 + D, hp, W + cs:W + ce],
                            start=True, stop=True,
                        )
                    Pex = at.tile([P, SW], bf16, tag="Pex", name="Pex")
                    nc.scalar.activation(Pex[:], psS[:, :SW], Act.Exp,
                                         scale=0.125 / (QSC * QSC))
                    # mask multiplies run out-of-place (keeps the DVE 2x
                    # bf16 mode), column-split across DVE and GpSimd so
                    # the masked strip is ready ~2x sooner
                    Pe = at.tile([P, SW], bf16, tag="Pe", name="Pe")
                    nc.vector.tensor_mul(Pe[:, :SW // 2], Pex[:, :SW // 2],
                                         maskF_sb[:, :SW // 2])
                    nc.gpsimd.tensor_mul(Pe[:, SW // 2:], Pex[:, SW // 2:],
                                         maskF_sb[:, SW // 2:])
                    Pes.append(Pe)
                cur = (yA, rs, Pes, hp)
            if prev is not None:
                pyA, prs, pPes, php = prev
                for s in (0, 1):
                    r0 = D * s
                    h = 2 * php + s
                    Pe = pPes[s]
                    for c in range(NKC):
                        off, cs = CHUNK_OFF[c], CHUNK_CS[c]
                        for (qlo, qhi, st) in SPLITS[c]:
                            pc = off + qlo - cs
                            nc.tensor.matmul(
                                pyA[r0:r0 + D, qlo:qhi],
                                v_sb[:, c, h * D:(h + 1) * D],
                                Pe[:, pc:pc + qhi - qlo],
                                start=st, stop=(s == 1 and c == NKC - 1),
                                tile_position=(0, r0), skip_group_check=True,
                            )
                    for c in range(NKC):
                        off, cs = CHUNK_OFF[c], CHUNK_CS[c]
                        for (qlo, qhi, st) in SPLITS[c]:
                            pc = off + qlo - cs
                            nc.tensor.matmul(
                                prs[r0:r0 + D, qlo:qhi],
                                voner_sb[:, c],
                                Pe[:, pc:pc + qhi - qlo],
                                start=st, stop=(s == 1 and c == NKC - 1),
                                tile_position=(0, r0), skip_group_check=True,
                            )
                rr = rrp.tile([P, QL], f32, tag="rr", name="rr")
                nc.vector.reciprocal_approx_fast(rr[:], prs[:])
                for s in (0, 1):
                    r0 = D * s
                    nc.vector.tensor_mul(yT_sb[r0:r0 + D, php, :],
                                         pyA[r0:r0 + D], rr[r0:r0 + D])
            prev = cur

        # ---- phase 3: out = y @ Wproj + b ----
        # bias lands as a rank-1 matmul (ones x bpr-row) and the PSUM
        # drain runs on the otherwise-idle scalar engine, keeping DVE
        # out of the slot-recycle path
        for tt in range(QL // P):
            for n0 in (0, QL):
                psf = psM.tile([P, 2 * QL], f32, tag="ps", name="ps3")
                ps = psf[:, :QL]
                for kc in range(KC):
                    nc.tensor.matmul(
                        ps, yT_sb[:, kc, tt * P:(tt + 1) * P],
                        wp_sb[:, kc, n0:n0 + QL],
                        start=(kc == 0), stop=False,
                    )
                nc.tensor.matmul(
                    ps, ones_row[:], bpr_sb[0:1, n0:n0 + QL],
                    start=False, stop=True,
                )
                osb = ot.tile([P, QL], f32, tag="osb", name="osb")
                nc.scalar.activation(osb[:], ps, Act.Copy)
                nc.sync.dma_start(out[tt * P:(tt + 1) * P, n0:n0 + QL], osb[:])


def _build():
    nc = bacc.Bacc(
        "TRN2", target_bir_lowering=False, debug=False,
        enable_asserts=True, num_devices=N_CORES,
    )
    fp8 = mybir.dt.float8e4
    xt = nc.dram_tensor("xt", [P, KC * HT], bf16, kind="ExternalInput").ap()
    xt8 = nc.dram_tensor("xt8", [P, KC * HT], fp8, kind="ExternalInput").ap()
    wqk8 = nc.dram_tensor("wqk8", [NFT // 2, P, 2 * KC * P], fp8,
                          kind="ExternalInput").ap()
    wv = nc.dram_tensor("wv", [C, C], bf16, kind="ExternalInput").ap()
    wp = nc.dram_tensor("wp", [C, C], bf16, kind="ExternalInput").ap()
    bqk = nc.dram_tensor("bqk", [2 * C], f32, kind="ExternalInput").ap()
    bvr = nc.dram_tensor("bvr", [P, C], bf16, kind="ExternalInput").ap()
    bvr0 = nc.dram_tensor("bvr0", [P, C], bf16, kind="ExternalInput").ap()
    bpr = nc.dram_tensor("bpr", [P, C], bf16, kind="ExternalInput").ap()
    maskF = nc.dram_tensor("maskF", [P, SW], bf16, kind="ExternalInput").ap()
    voner = nc.dram_tensor("voner", [HTP, D], bf16, kind="ExternalInput").ap()
    out = nc.dram_tensor("out", [QL, C], f32, kind="ExternalOutput").ap()
    with tile.TileContext(nc) as tc:
        _emit(tc, xt, xt8, wqk8, wv, wp, bqk, bvr, bvr0, bpr, maskF, voner, out)
    nc.compile()
    return nc


def _get_module():
    if "nc" not in _CACHE:
        _CACHE["nc"] = _build()
    return _CACHE["nc"]


def _band_mask() -> np.ndarray:
    # strip col off+ (q-cs) for chunk c, partition p = local key 128c+p;
    # query qq visible to key kk iff qq <= kk <= qq+64 (halo frame)
    m = np.zeros((P, SW), dtype=np.float32)
    p = np.arange(P)
    for c in range(NKC):
        off, cs, ce = CHUNK_OFF[c], CHUNK_CS[c], CHUNK_CE[c]
        qq = np.arange(cs, ce)[None, :]
        kk = (128 * c + p)[:, None]
        m[:, off:off + ce - cs] = (kk >= qq) & (kk <= qq + W)
    return m.astype(ml_dtypes.bfloat16)


def kernel(x, Wqkv, bqkv, Wproj, bproj):
    x = np.asarray(x, dtype=np.float32)
    Wqkv = np.asarray(Wqkv, dtype=np.float32)
    bqkv = np.asarray(bqkv, dtype=np.float32)
    Wproj = np.asarray(Wproj, dtype=np.float32)
    bproj = np.asarray(bproj, dtype=np.float32)

    bf = ml_dtypes.bfloat16
    f8 = ml_dtypes.float8_e4m3
    # pack Wqk slab-major in the exact SBUF tile layout: [ft, p, kc*128],
    # pre-scaled by QSC so the fp8 values sit in e4m3's sweet spot
    wqk8_np = np.ascontiguousarray(
        (Wqkv[:, :2 * C] * QSC).astype(f8).reshape(KC, P, NFT // 2, 2, P)
        .transpose(2, 1, 3, 0, 4).reshape(NFT // 2, P, 2 * KC * P)
    )
    wv_np = np.ascontiguousarray(Wqkv[:, 2 * C:]).astype(bf)
    wp_np = Wproj.astype(bf)
    bqk_np = np.ascontiguousarray(bqkv[:2 * C] * QSC)
    bvr_np = np.ascontiguousarray(np.broadcast_to(bqkv[2 * C:], (P, C))).astype(bf)
    bvr0_np = bvr_np.copy()
    bvr0_np[:W] = 0.0  # halo-pad tokens of the q==0 cores carry no bias
    bpr_np = np.ascontiguousarray(np.broadcast_to(bproj, (P, C))).astype(bf)
    maskF_np = _band_mask()

    vone_real = np.zeros((HTP, D), dtype=bf)
    vone_real[:HT] = 1.0
    vone_pad = vone_real.copy()
    vone_pad[:W] = 0.0

    in_maps = _prep_in_maps(x, wqk8_np, wv_np, wp_np, bqk_np, bvr_np, bvr0_np,
                            bpr_np, maskF_np, vone_real, vone_pad)

    nc = _get_module()
    _CACHE["last_in_maps"] = in_maps
    res = bass_utils.run_bass_kernel_spmd(nc, in_maps, core_ids=list(range(N_CORES)))

    out = np.empty((B, T, C), dtype=np.float32)
    for c in range(N_CORES):
        b, q = divmod(c, 4)
        out[b, q * QL:(q + 1) * QL] = res.results[c]["out"]
    return out


def _prep_in_maps(x, wqk8_np, wv_np, wp_np, bqk_np, bvr_np, bvr0_np, bpr_np,
                  maskF_np, vone_real, vone_pad):
    bf = ml_dtypes.bfloat16
    f8 = ml_dtypes.float8_e4m3
    in_maps = []
    for c in range(N_CORES):
        b, q = divmod(c, 4)
        lo = q * QL - W
        if lo < 0:
            chunk = np.concatenate(
                [np.zeros((W, C), np.float32), x[b, 0:q * QL + QL]], axis=0
            )
        else:
            chunk = x[b, lo:lo + HT]
        # pack x^T in SBUF tile order: [p, kc*576]
        xtt = chunk.T.reshape(KC, P, HT).transpose(1, 0, 2).reshape(P, KC * HT)
        xt_np = np.ascontiguousarray(xtt.astype(bf))
        xt8_np = np.ascontiguousarray(xtt.astype(f8))
        in_maps.append({
            "xt": xt_np,
            "xt8": xt8_np,
            "wqk8": wqk8_np,
            "wv": wv_np,
            "wp": wp_np,
            "bqk": bqk_np,
            "bvr": bvr_np,
            "bvr0": bvr0_np if q == 0 else bvr_np,
            "bpr": bpr_np,
            "maskF": maskF_np,
            "voner": vone_pad if q == 0 else vone_real,
        })
    return in_maps


# revision 43
# speedup vs baseline: 1.0112x; 1.0112x over previous
"""Banded causal self-attention (band width 64) on 8 trn2 NeuronCores.

Sequence-parallel sharding: core c handles batch c//4, query block c%4
(512 queries of T=2048), recomputing a 64-token k/v halo locally so no
collectives are needed. The host casts inputs to bf16 and transposes x
per core; the device kernel fuses qkv-projection -> banded attention ->
output projection.

Device layouts (per core):
  xt/xt8  [128, 8*576]  x chunk transposed, host-packed in SBUF tile
                        order (bf16 for v, fp8e4m3 for q/k)
  qk^T    [2048, 640]   q/k feature-major; key cols 576:640 zero-padded
                        so every key chunk is a full 128 partitions
  v       [640, 1024]   token-major (rows 576:640 zeroed)
  y^T     [1024, 512]   attention output feature-major
  out     [512, 1024]   tokens x C

The q/k projection runs in fp8e4m3 with DoubleRow perf mode (weights
pre-scaled x64; the exp rescales scores by 0.125/64^2) - q/k errors
are damped by the score scale + softmax, unlike v/proj which stay
bf16. Attention is computed transposed (S^T[key, query]) with all 5
key chunks of a (head, s) pair batched into one [128, 768] PSUM strip,
so a single Exp activation covers the strip; the mask multiplies run
out-of-place, split across DVE (s=0) and GpSimd (s=1). Rowsums come
from voner-stationary matmuls replicated over 64 partitions; their
reciprocal uses the fast approx DVE op. AV/rowsum accumulation uses
per-region start flags (no zero-priming matmuls), and the whole
attention loop is software-pipelined by one head pair. All heavy DMAs
ride the in-order sync queue, host-packed for multi-KB contiguous
DRAM runs and batched so one dma_start issue (~590ns of queue time)
covers ~2.4us of PE work. Softmax skips max-subtraction (scores are
O(1)).
"""

import numpy as np
import ml_dtypes

import concourse.mybir as mybir
import concourse.tile as tile
from concourse import bacc
from concourse import bass_utils

B, T, C, H, D = 2, 2048, 1024, 16, 64
W = 64            # band width: key j visible to query i iff i-64 <= j <= i
N_CORES = 8
QL = 512          # queries per core
HT = QL + W       # tokens incl. halo
HTP = QL + 2 * W  # key columns incl. zero pad (640)
P = 128
KC = C // P       # contraction chunks
NFT = 2 * C // P  # q|k feature slabs
NKC = 5           # key chunks (5 x 128, last half zero-padded)
SW = 768          # batched score-strip width (sum of chunk windows)

QSC = 64.0        # fp8 pre-scale on Wqk (q/k come out x64; exp rescales)

bf16 = mybir.dt.bfloat16
f32 = mybir.dt.float32
Act = mybir.ActivationFunctionType

_CACHE = {}

# per key chunk: column offset in the score strip, query window [cs, ce)
CHUNK_OFF = [0, 128, 320, 512, 704]
CHUNK_CS = [0, 64, 192, 320, 448]
CHUNK_CE = [128, 256, 384, 512, 512]
# AV/rowsum accumulation pieces per chunk: (qlo, qhi, first_writer)
SPLITS = [
    [(0, 128, True)],
    [(64, 128, False), (128, 256, True)],
    [(192, 256, False), (256, 384, True)],
    [(320, 384, False), (384, 512, True)],
    [(448, 512, False)],
]


def _emit(tc, xt, xt8, wqk8, wv, wp, bqk, bvr, bvr0, bpr, maskF, voner, out):
    nc = tc.nc
    with (
        tc.tile_pool(name="const", bufs=1) as const,
        tc.tile_pool(name="wqkp", bufs=3) as wqkp,
        tc.tile_pool(name="attn", bufs=4) as at,
        tc.tile_pool(name="rrp", bufs=2) as rrp,
        tc.tile_pool(name="ot", bufs=3) as ot,
        tc.tile_pool(name="psM", bufs=2, space="PSUM") as psM,
        tc.tile_pool(name="psY", bufs=2, space="PSUM") as psY,
        tc.tile_pool(name="psR", bufs=2, space="PSUM") as psR,
    ):
        # ---- persistent tiles; critical-path DMAs first ----
        # xt/wqk arrive host-packed in SBUF tile order so every DMA runs
        # multi-KB contiguous DRAM reads (256B runs measured ~1/3 peak).
        # All heavy transfers go on the in-order sync queue so the tile
        # scheduler cannot hoist them ahead of the hot wqk slabs; the
        # scalar HWDGE queue only carries the small tiles.
        xt8_sb = const.tile([P, KC, HT], mybir.dt.float8e4)
        xt8_r = xt8.rearrange("p (kc t) -> p kc t", t=HT)
        nc.sync.dma_start(xt8_sb[:, : KC // 2], xt8_r[:, : KC // 2])
        nc.sync.dma_start(xt8_sb[:, KC // 2 :], xt8_r[:, KC // 2 :])
        bqk_sb = const.tile([P, NFT], f32)
        nc.scalar.dma_start(bqk_sb[:], bqk.rearrange("(ft p) -> p ft", p=P))

        xt_sb = const.tile([P, KC, HT], bf16)
        xt_r = xt.rearrange("p (kc t) -> p kc t", t=HT)
        wv_sb = const.tile([P, KC, C], bf16)
        wv_r = wv.rearrange("(kc p) n -> p kc n", p=P)
        wp_sb = const.tile([P, KC, C], bf16)
        wp_r = wp.rearrange("(kc p) n -> p kc n", p=P)
        maskF_sb = const.tile([P, SW], bf16)
        bvr_sb = const.tile([P, C], bf16)
        bvr0_sb = const.tile([P, C], bf16)
        bpr_sb = const.tile([P, C], bf16)
        voner_sb = const.tile([P, NKC, D], bf16)
        nc.scalar.dma_start(bvr_sb[:], bvr[:])
        nc.scalar.dma_start(bvr0_sb[:], bvr0[:])
        nc.scalar.dma_start(maskF_sb[:], maskF[:])
        nc.scalar.dma_start(voner_sb[:], voner.rearrange("(c p) e -> p c e", p=P))
        nc.scalar.dma_start(bpr_sb[:], bpr[:])

        qkT_sb = const.tile([P, NFT, HTP], bf16)
        nc.gpsimd.memset(qkT_sb[:, :, HT:HTP], 0.0)  # zero key pad columns
        v_sb = const.tile([P, NKC, C], bf16)
        nc.gpsimd.memset(v_sb[W:P, NKC - 1], 0.0)    # zero fake-token rows
        yT_sb = const.tile([P, KC, QL], bf16)

        # ---- phase 1a: qk^T = Wqk8^T @ x8^T (fp8 DoubleRow, feature-major) --
        # slab DMAs are batched in pairs: one ~590ns dma_start issue per
        # ~2.4us of PE work, so the sync queue's issue rate can keep ahead
        DR = mybir.MatmulPerfMode.DoubleRow
        for fp in range(NFT // 2):
            wt = wqkp.tile([P, 2, KC, P], mybir.dt.float8e4, tag="wqk")
            nc.sync.dma_start(
                wt[:],
                wqk8[fp].rearrange("p (two kc f) -> p two kc f", two=2, f=P),
            )
            # x (bf16, for v) and Wv ride along behind the hot wqk pairs,
            # spread thin enough that no pair transfer is ever delayed
            if 1 <= fp <= 4:
                qs = slice(2 * (fp - 1), 2 * fp)
                nc.sync.dma_start(xt_sb[:, qs], xt_r[:, qs])
            if 4 <= fp < 8:
                c2 = 2 * (fp - 4)
                nc.sync.dma_start(wv_sb[:, c2:c2 + 2], wv_r[:, c2:c2 + 2])
            for half in (0, 1):
                ft = 2 * fp + half
                # q is only needed for owned tokens (64:576); k needs all
                # 576, written as two segment regions of ONE psum tile so
                # each slab costs a single slot + a single activation
                segs = ((W, QL),) if ft < KC else ((QL, W), (0, QL))
                psf = psM.tile([P, 2 * QL], f32, tag="ps", name="ps1a")
                for t0, tsz in segs:
                    ps = psf[:, t0:t0 + tsz] if ft >= KC else psf[:, :tsz]
                    for kp in range(KC // 2):
                        nc.tensor.matmul(
                            ps, wt[:, half, 2 * kp:2 * kp + 2],
                            xt8_sb[:, 2 * kp:2 * kp + 2, t0:t0 + tsz],
                            start=(kp == 0), stop=(kp == KC // 2 - 1),
                            perf_mode=DR,
                        )
                if ft < KC:
                    nc.scalar.activation(
                        qkT_sb[:, ft, W:W + QL], psf[:, :QL], Act.Identity,
                        bias=bqk_sb[:, ft:ft + 1],
                    )
                else:
                    nc.scalar.activation(
                        qkT_sb[:, ft, :HT], psf[:, :HT], Act.Identity,
                        bias=bqk_sb[:, ft:ft + 1],
                    )

        for c2 in range(0, KC, 2):  # wp behind wv, still ordered on sync
            nc.sync.dma_start(wp_sb[:, c2:c2 + 2], wp_r[:, c2:c2 + 2])

        # ---- phase 1b: v = x @ Wv (token-major) ----
        for tt in range(NKC):
            tsz = P if tt < NKC - 1 else W
            bsel = bvr0_sb if tt == 0 else bvr_sb
            for n0 in (0, QL):
                psf = psM.tile([P, 2 * QL], f32, tag="ps", name="ps1b")
                ps = psf[:tsz, :QL]
                for kc in range(KC):
                    nc.tensor.matmul(
                        ps, xt_sb[:, kc, tt * P:tt * P + tsz],
                        wv_sb[:, kc, n0:n0 + QL],
                        start=(kc == 0), stop=(kc == KC - 1),
                    )
                nc.vector.tensor_add(
                    v_sb[:tsz, tt, n0:n0 + QL], ps, bsel[:tsz, n0:n0 + QL],
                )

        # ---- phase 2: banded attention, transposed-S form ----
        # software-pipelined by one head pair: scores/exp/mask of hp are
        # emitted before the AV/rowsum/normalize of hp-1, so the PE streams
        # the next pair's scores while ACT/GpSimd work on the current one.
        prev = None
        for hp in range(H // 2 + 1):
            cur = None
            if hp < H // 2:
                yA = psY.tile([P, QL], f32, tag="yA")
                rs = psR.tile([P, QL], f32, tag="rs")
                Pes = []
                for s in (0, 1):
                    r0 = D * s
                    psS = psM.tile([P, 2 * QL], f32, tag="ps", name="psS")
                    for c in range(NKC):
                        off, cs, ce = CHUNK_OFF[c], CHUNK_CS[c], CHUNK_CE[c]
                        nc.tensor.matmul(
                            psS[:, off:off + ce - cs],
                            qkT_sb[r0:r0 + D, KC + hp, c * P:(c + 1) * P],
                            qkT_sb[r0:r0 + D, hp, W + cs:W + ce],
                            start=True, stop=True,
                        )
                    Pex = at.tile([P, SW], bf16, tag="Pex", name="Pex")
                    nc.scalar.activation(Pex[:], psS[:, :SW], Act.Exp,
                                         scale=0.125 / (QSC * QSC))
                    # mask multiplies run out-of-place (keeps the DVE 2x
                    # bf16 mode), column-split across DVE and GpSimd so
                    # the masked strip is ready ~2x sooner
                    Pe = at.tile([P, SW], bf16, tag="Pe", name="Pe")
                    nc.vector.tensor_mul(Pe[:, :SW // 2], Pex[:, :SW // 2],
                                         maskF_sb[:, :SW // 2])
                    nc.gpsimd.tensor_mul(Pe[:, SW // 2:], Pex[:, SW // 2:],
                                         maskF_sb[:, SW // 2:])
                    Pes.append(Pe)
                cur = (yA, rs, Pes, hp)
            if prev is not None:
                pyA, prs, pPes, php = prev
                for s in (0, 1):
                    r0 = D * s
                    h = 2 * php + s
                    Pe = pPes[s]
                    for c in range(NKC):
                        off, cs = CHUNK_OFF[c], CHUNK_CS[c]
                        for (qlo, qhi, st) in SPLITS[c]:
                            pc = off + qlo - cs
                            nc.tensor.matmul(
                                pyA[r0:r0 + D, qlo:qhi],
                                v_sb[:, c, h * D:(h + 1) * D],
                                Pe[:, pc:pc + qhi - qlo],
                                start=st, stop=(s == 1 and c == NKC - 1),
                                tile_position=(0, r0), skip_group_check=True,
                            )
                    for c in range(NKC):
                        off, cs = CHUNK_OFF[c], CHUNK_CS[c]
                        for (qlo, qhi, st) in SPLITS[c]:
                            pc = off + qlo - cs
                            nc.tensor.matmul(
                                prs[r0:r0 + D, qlo:qhi],
                                voner_sb[:, c],
                                Pe[:, pc:pc + qhi - qlo],
                                start=st, stop=(s == 1 and c == NKC - 1),
                                tile_position=(0, r0), skip_group_check=True,
                            )
                rr = rrp.tile([P, QL], f32, tag="rr", name="rr")
                nc.vector.reciprocal_approx_fast(rr[:], prs[:])
                for s in (0, 1):
                    r0 = D * s
                    nc.vector.tensor_mul(yT_sb[r0:r0 + D, php, :],
                                         pyA[r0:r0 + D], rr[r0:r0 + D])
            prev = cur

        # ---- phase 3: out = y @ Wproj + b ----
        for tt in range(QL // P):
            for n0 in (0, QL):
                psf = psM.tile([P, 2 * QL], f32, tag="ps", name="ps3")
                ps = psf[:, :QL]
                for kc in range(KC):
                    nc.tensor.matmul(
                        ps, yT_sb[:, kc, tt * P:(tt + 1) * P],
                        wp_sb[:, kc, n0:n0 + QL],
                        start=(kc == 0), stop=(kc == KC - 1),
                    )
                osb = ot.tile([P, QL], f32, tag="osb", name="osb")
                nc.vector.tensor_add(osb[:], ps, bpr_sb[:, n0:n0 + QL])
                nc.sync.dma_start(out[tt * P:(tt + 1) * P, n0:n0 + QL], osb[:])


def _build():
    nc = bacc.Bacc(
        "TRN2", target_bir_lowering=False, debug=False,
        enable_asserts=True, num_devices=N_CORES,
    )
    fp8 = mybir.dt.float8e4
    xt = nc.dram_tensor("xt", [P, KC * HT], bf16, kind="ExternalInput").ap()
    xt8 = nc.dram_tensor("xt8", [P, KC * HT], fp8, kind="ExternalInput").ap()
    wqk8 = nc.dram_tensor("wqk8", [NFT // 2, P, 2 * KC * P], fp8,
                          kind="ExternalInput").ap()
    wv = nc.dram_tensor("wv", [C, C], bf16, kind="ExternalInput").ap()
    wp = nc.dram_tensor("wp", [C, C], bf16, kind="ExternalInput").ap()
    bqk = nc.dram_tensor("bqk", [2 * C], f32, kind="ExternalInput").ap()
    bvr = nc.dram_tensor("bvr", [P, C], bf16, kind="ExternalInput").ap()
    bvr0 = nc.dram_tensor("bvr0", [P, C], bf16, kind="ExternalInput").ap()
    bpr = nc.dram_tensor("bpr", [P, C], bf16, kind="ExternalInput").ap()
    maskF = nc.dram_tensor("maskF", [P, SW], bf16, kind="ExternalInput").ap()
    voner = nc.dram_tensor("voner", [HTP, D], bf16, kind="ExternalInput").ap()
    out = nc.dram_tensor("out", [QL, C], f32, kind="ExternalOutput").ap()
    with tile.TileContext(nc) as tc:
        _emit(tc, xt, xt8, wqk8, wv, wp, bqk, bvr, bvr0, bpr, maskF, voner, out)
    nc.compile()
    return nc


def _get_module():
    if "nc" not in _CACHE:
        _CACHE["nc"] = _build()
    return _CACHE["nc"]


def _band_mask() -> np.ndarray:
    # strip col off+ (q-cs) for chunk c, partition p = local key 128c+p;
    # query qq visible to key kk iff qq <= kk <= qq+64 (halo frame)
    m = np.zeros((P, SW), dtype=np.float32)
    p = np.arange(P)
    for c in range(NKC):
        off, cs, ce = CHUNK_OFF[c], CHUNK_CS[c], CHUNK_CE[c]
        qq = np.arange(cs, ce)[None, :]
        kk = (128 * c + p)[:, None]
        m[:, off:off + ce - cs] = (kk >= qq) & (kk <= qq + W)
    return m.astype(ml_dtypes.bfloat16)


def kernel(x, Wqkv, bqkv, Wproj, bproj):
    x = np.asarray(x, dtype=np.float32)
    Wqkv = np.asarray(Wqkv, dtype=np.float32)
    bqkv = np.asarray(bqkv, dtype=np.float32)
    Wproj = np.asarray(Wproj, dtype=np.float32)
    bproj = np.asarray(bproj, dtype=np.float32)

    bf = ml_dtypes.bfloat16
    f8 = ml_dtypes.float8_e4m3
    # pack Wqk slab-major in the exact SBUF tile layout: [ft, p, kc*128],
    # pre-scaled by QSC so the fp8 values sit in e4m3's sweet spot
    wqk8_np = np.ascontiguousarray(
        (Wqkv[:, :2 * C] * QSC).astype(f8).reshape(KC, P, NFT // 2, 2, P)
        .transpose(2, 1, 3, 0, 4).reshape(NFT // 2, P, 2 * KC * P)
    )
    wv_np = np.ascontiguousarray(Wqkv[:, 2 * C:]).astype(bf)
    wp_np = Wproj.astype(bf)
    bqk_np = np.ascontiguousarray(bqkv[:2 * C] * QSC)
    bvr_np = np.ascontiguousarray(np.broadcast_to(bqkv[2 * C:], (P, C))).astype(bf)
    bvr0_np = bvr_np.copy()
    bvr0_np[:W] = 0.0  # halo-pad tokens of the q==0 cores carry no bias
    bpr_np = np.ascontiguousarray(np.broadcast_to(bproj, (P, C))).astype(bf)
    maskF_np = _band_mask()

    vone_real = np.zeros((HTP, D), dtype=bf)
    vone_real[:HT] = 1.0
    vone_pad = vone_real.copy()
    vone_pad[:W] = 0.0

    in_maps = _prep_in_maps(x, wqk8_np, wv_np, wp_np, bqk_np, bvr_np, bvr0_np,
                            bpr_np, maskF_np, vone_real, vone_pad)

    nc = _get_module()
    _CACHE["last_in_maps"] = in_maps
    res = bass_utils.run_bass_kernel_spmd(nc, in_maps, core_ids=list(range(N_CORES)))

    out = np.empty((B, T, C), dtype=np.float32)
    for c in range(N_CORES):
        b, q = divmod(c, 4)
        out[b, q * QL:(q + 1) * QL] = res.results[c]["out"]
    return out


def _prep_in_maps(x, wqk8_np, wv_np, wp_np, bqk_np, bvr_np, bvr0_np, bpr_np,
                  maskF_np, vone_real, vone_pad):
    bf = ml_dtypes.bfloat16
    f8 = ml_dtypes.float8_e4m3
    in_maps = []
    for c in range(N_CORES):
        b, q = divmod(c, 4)
        lo = q * QL - W
        if lo < 0:
            chunk = np.concatenate(
                [np.zeros((W, C), np.float32), x[b, 0:q * QL + QL]], axis=0
            )
        else:
            chunk = x[b, lo:lo + HT]
        # pack x^T in SBUF tile order: [p, kc*576]
        xtt = chunk.T.reshape(KC, P, HT).transpose(1, 0, 2).reshape(P, KC * HT)
        xt_np = np.ascontiguousarray(xtt.astype(bf))
        xt8_np = np.ascontiguousarray(xtt.astype(f8))
        in_maps.append({
            "xt": xt_np,
            "xt8": xt8_np,
            "wqk8": wqk8_np,
            "wv": wv_np,
            "wp": wp_np,
            "bqk": bqk_np,
            "bvr": bvr_np,
            "bvr0": bvr0_np if q == 0 else bvr_np,
            "bpr": bpr_np,
            "maskF": maskF_np,
            "voner": vone_pad if q == 0 else vone_real,
        })
    return in_maps


# revision 44
# speedup vs baseline: 1.0865x; 1.0746x over previous
"""Banded causal self-attention (band width 64) on 8 trn2 NeuronCores.

Sequence-parallel sharding: core c handles batch c//4, query block c%4
(512 queries of T=2048), recomputing a 64-token k/v halo locally so no
collectives are needed. The host casts inputs to bf16 and transposes x
per core; the device kernel fuses qkv-projection -> banded attention ->
output projection.

Device layouts (per core):
  xt/xt8  [128, 8*576]  x chunk transposed, host-packed in SBUF tile
                        order (bf16 for v, fp8e4m3 for q/k)
  qk^T    [2048, 640]   q/k feature-major; key cols 576:640 zero-padded
                        so every key chunk is a full 128 partitions
  v       [640, 1024]   token-major (rows 576:640 zeroed)
  y^T     [1024, 512]   attention output feature-major
  out     [512, 1024]   tokens x C

The q/k projection runs in fp8e4m3 with DoubleRow perf mode (weights
pre-scaled x64; the exp rescales scores by 0.125/64^2) - q/k errors
are damped by the score scale + softmax, unlike v/proj which stay
bf16. Attention is computed transposed (S^T[key, query]) with all 5
key chunks of a (head, s) pair batched into one [128, 768] PSUM strip,
so a single Exp activation covers the strip; the mask multiplies run
out-of-place, split across DVE (s=0) and GpSimd (s=1). Rowsums come
from voner-stationary matmuls replicated over 64 partitions; their
reciprocal uses the fast approx DVE op. AV/rowsum accumulation uses
per-region start flags (no zero-priming matmuls), and the whole
attention loop is software-pipelined by one head pair. All heavy DMAs
ride the in-order sync queue, host-packed for multi-KB contiguous
DRAM runs and batched so one dma_start issue (~590ns of queue time)
covers ~2.4us of PE work. Softmax skips max-subtraction (scores are
O(1)).
"""

import numpy as np
import ml_dtypes

import concourse.mybir as mybir
import concourse.tile as tile
from concourse import bacc
from concourse import bass_utils

B, T, C, H, D = 2, 2048, 1024, 16, 64
W = 64            # band width: key j visible to query i iff i-64 <= j <= i
N_CORES = 8
QL = 512          # queries per core
HT = QL + W       # tokens incl. halo
HTP = QL + 2 * W  # key columns incl. zero pad (640)
P = 128
KC = C // P       # contraction chunks
NFT = 2 * C // P  # q|k feature slabs
NKC = 5           # key chunks (5 x 128, last half zero-padded)
SW = 768          # batched score-strip width (sum of chunk windows)

QSC = 64.0        # fp8 pre-scale on Wqk (q/k come out x64; exp rescales)

bf16 = mybir.dt.bfloat16
f32 = mybir.dt.float32
Act = mybir.ActivationFunctionType

_CACHE = {}

# per key chunk: column offset in the score strip, query window [cs, ce)
CHUNK_OFF = [0, 128, 320, 512, 704]
CHUNK_CS = [0, 64, 192, 320, 448]
CHUNK_CE = [128, 256, 384, 512, 512]
# AV/rowsum accumulation pieces per chunk: (qlo, qhi, first_writer)
SPLITS = [
    [(0, 128, True)],
    [(64, 128, False), (128, 256, True)],
    [(192, 256, False), (256, 384, True)],
    [(320, 384, False), (384, 512, True)],
    [(448, 512, False)],
]


def _emit(tc, xt, xt8, wqk8, wv, wp, bqk, bvr, bvr0, bpr, maskF, voner, out):
    nc = tc.nc
    with (
        tc.tile_pool(name="const", bufs=1) as const,
        tc.tile_pool(name="wqkp", bufs=3) as wqkp,
        tc.tile_pool(name="attn", bufs=4) as at,
        tc.tile_pool(name="rrp", bufs=2) as rrp,
        tc.tile_pool(name="ot", bufs=3) as ot,
        tc.tile_pool(name="psM", bufs=2, space="PSUM") as psM,
        tc.tile_pool(name="psY", bufs=2, space="PSUM") as psY,
        tc.tile_pool(name="psR", bufs=2, space="PSUM") as psR,
    ):
        # ---- persistent tiles; critical-path DMAs first ----
        # xt/wqk arrive host-packed in SBUF tile order so every DMA runs
        # multi-KB contiguous DRAM reads (256B runs measured ~1/3 peak).
        # All heavy transfers go on the in-order sync queue so the tile
        # scheduler cannot hoist them ahead of the hot wqk slabs; the
        # scalar HWDGE queue only carries the small tiles.
        xt8_sb = const.tile([P, KC, HT], mybir.dt.float8e4)
        xt8_r = xt8.rearrange("p (kc t) -> p kc t", t=HT)
        nc.sync.dma_start(xt8_sb[:, : KC // 2], xt8_r[:, : KC // 2])
        nc.sync.dma_start(xt8_sb[:, KC // 2 :], xt8_r[:, KC // 2 :])
        bqk_sb = const.tile([P, NFT], f32)
        nc.scalar.dma_start(bqk_sb[:], bqk.rearrange("(ft p) -> p ft", p=P))

        xt_sb = const.tile([P, KC, HT], bf16)
        xt_r = xt.rearrange("p (kc t) -> p kc t", t=HT)
        wv_sb = const.tile([P, KC, C], bf16)
        wv_r = wv.rearrange("(kc p) n -> p kc n", p=P)
        wp_sb = const.tile([P, KC, C], bf16)
        wp_r = wp.rearrange("(kc p) n -> p kc n", p=P)
        maskF_sb = const.tile([P, SW], bf16)
        bvr_sb = const.tile([P, C], bf16)
        bvr0_sb = const.tile([P, C], bf16)
        bpr_sb = const.tile([P, C], bf16)
        voner_sb = const.tile([P, NKC, D], bf16)
        nc.scalar.dma_start(bvr_sb[:], bvr[:])
        nc.scalar.dma_start(bvr0_sb[:], bvr0[:])
        nc.scalar.dma_start(maskF_sb[:], maskF[:])
        nc.scalar.dma_start(voner_sb[:], voner.rearrange("(c p) e -> p c e", p=P))
        nc.scalar.dma_start(bpr_sb[:], bpr[:])

        qkT_sb = const.tile([P, NFT, HTP], bf16)
        nc.gpsimd.memset(qkT_sb[:, :, HT:HTP], 0.0)  # zero key pad columns
        v_sb = const.tile([P, NKC, C], bf16)
        nc.gpsimd.memset(v_sb[W:P, NKC - 1], 0.0)    # zero fake-token rows
        yT_sb = const.tile([P, KC, QL], bf16)

        # ---- phase 1a: qk^T = Wqk8^T @ x8^T (fp8 DoubleRow, feature-major) --
        # slab DMAs are batched in pairs: one ~590ns dma_start issue per
        # ~2.4us of PE work, so the sync queue's issue rate can keep ahead
        DR = mybir.MatmulPerfMode.DoubleRow
        for fp in range(NFT // 2):
            wt = wqkp.tile([P, 2, KC, P], mybir.dt.float8e4, tag="wqk")
            nc.sync.dma_start(
                wt[:],
                wqk8[fp].rearrange("p (two kc f) -> p two kc f", two=2, f=P),
            )
            # x (bf16, for v) and Wv ride along behind the hot wqk pairs,
            # spread thin enough that no pair transfer is ever delayed
            if 1 <= fp <= 4:
                qs = slice(2 * (fp - 1), 2 * fp)
                nc.sync.dma_start(xt_sb[:, qs], xt_r[:, qs])
            if 4 <= fp < 8:
                c2 = 2 * (fp - 4)
                nc.sync.dma_start(wv_sb[:, c2:c2 + 2], wv_r[:, c2:c2 + 2])
            for half in (0, 1):
                ft = 2 * fp + half
                # q is only needed for owned tokens (64:576); k needs all
                # 576, written as two segment regions of ONE psum tile so
                # each slab costs a single slot + a single activation
                segs = ((W, QL),) if ft < KC else ((QL, W), (0, QL))
                psf = psM.tile([P, 2 * QL], f32, tag="ps", name="ps1a")
                for t0, tsz in segs:
                    ps = psf[:, t0:t0 + tsz] if ft >= KC else psf[:, :tsz]
                    for kp in range(KC // 2):
                        nc.tensor.matmul(
                            ps, wt[:, half, 2 * kp:2 * kp + 2],
                            xt8_sb[:, 2 * kp:2 * kp + 2, t0:t0 + tsz],
                            start=(kp == 0), stop=(kp == KC // 2 - 1),
                            perf_mode=DR,
                        )
                if ft < KC:
                    nc.scalar.activation(
                        qkT_sb[:, ft, W:W + QL], psf[:, :QL], Act.Identity,
                        bias=bqk_sb[:, ft:ft + 1],
                    )
                else:
                    nc.scalar.activation(
                        qkT_sb[:, ft, :HT], psf[:, :HT], Act.Identity,
                        bias=bqk_sb[:, ft:ft + 1],
                    )

        for c2 in range(0, KC, 2):  # wp behind wv, still ordered on sync
            nc.sync.dma_start(wp_sb[:, c2:c2 + 2], wp_r[:, c2:c2 + 2])

        # ---- phase 1b: v = x @ Wv (token-major) ----
        for tt in range(NKC):
            tsz = P if tt < NKC - 1 else W
            bsel = bvr0_sb if tt == 0 else bvr_sb
            for n0 in (0, QL):
                psf = psM.tile([P, 2 * QL], f32, tag="ps", name="ps1b")
                ps = psf[:tsz, :QL]
                for kc in range(KC):
                    nc.tensor.matmul(
                        ps, xt_sb[:, kc, tt * P:tt * P + tsz],
                        wv_sb[:, kc, n0:n0 + QL],
                        start=(kc == 0), stop=(kc == KC - 1),
                    )
                nc.vector.tensor_add(
                    v_sb[:tsz, tt, n0:n0 + QL], ps, bsel[:tsz, n0:n0 + QL],
                )

        # ---- phase 2: banded attention, transposed-S form ----
        # software-pipelined by one head pair: scores/exp/mask of hp are
        # emitted before the AV/rowsum/normalize of hp-1, so the PE streams
        # the next pair's scores while ACT/GpSimd work on the current one.
        prev = None
        for hp in range(H // 2 + 1):
            cur = None
            if hp < H // 2:
                yA = psY.tile([P, QL], f32, tag="yA")
                rs = psR.tile([P, QL], f32, tag="rs")
                Pes = []
                for s in (0, 1):
                    r0 = D * s
                    psS = psM.tile([P, 2 * QL], f32, tag="ps", name="psS")
                    for c in range(NKC):
                        off, cs, ce = CHUNK_OFF[c], CHUNK_CS[c], CHUNK_CE[c]
                        nc.tensor.matmul(
                            psS[:, off:off + ce - cs],
                            qkT_sb[r0:r0 + D, KC + hp, c * P:(c + 1) * P],
                            qkT_sb[r0:r0 + D, hp, W + cs:W + ce],
                            start=True, stop=True,
                        )
                    Pex = at.tile([P, SW], bf16, tag="Pex", name="Pex")
                    nc.scalar.activation(Pex[:], psS[:, :SW], Act.Exp,
                                         scale=0.125 / (QSC * QSC))
                    # mask multiplies run out-of-place (keeps the DVE 2x
                    # bf16 mode) and split across DVE/GpSimd so the two
                    # s-halves mask concurrently
                    Pe = at.tile([P, SW], bf16, tag="Pe", name="Pe")
                    meng = nc.vector if s == 0 else nc.gpsimd
                    meng.tensor_mul(Pe[:], Pex[:], maskF_sb[:])
                    Pes.append(Pe)
                cur = (yA, rs, Pes, hp)
            if prev is not None:
                pyA, prs, pPes, php = prev
                for s in (0, 1):
                    r0 = D * s
                    h = 2 * php + s
                    Pe = pPes[s]
                    for c in range(NKC):
                        off, cs = CHUNK_OFF[c], CHUNK_CS[c]
                        for (qlo, qhi, st) in SPLITS[c]:
                            pc = off + qlo - cs
                            nc.tensor.matmul(
                                pyA[r0:r0 + D, qlo:qhi],
                                v_sb[:, c, h * D:(h + 1) * D],
                                Pe[:, pc:pc + qhi - qlo],
                                start=st, stop=(s == 1 and c == NKC - 1),
                                tile_position=(0, r0), skip_group_check=True,
                            )
                    for c in range(NKC):
                        off, cs = CHUNK_OFF[c], CHUNK_CS[c]
                        for (qlo, qhi, st) in SPLITS[c]:
                            pc = off + qlo - cs
                            nc.tensor.matmul(
                                prs[r0:r0 + D, qlo:qhi],
                                voner_sb[:, c],
                                Pe[:, pc:pc + qhi - qlo],
                                start=st, stop=(s == 1 and c == NKC - 1),
                                tile_position=(0, r0), skip_group_check=True,
                            )
                rr = rrp.tile([P, QL], f32, tag="rr", name="rr")
                nc.vector.reciprocal_approx_fast(rr[:], prs[:])
                for s in (0, 1):
                    r0 = D * s
                    nc.vector.tensor_mul(yT_sb[r0:r0 + D, php, :],
                                         pyA[r0:r0 + D], rr[r0:r0 + D])
            prev = cur

        # ---- phase 3: out = y @ Wproj + b ----
        for tt in range(QL // P):
            for n0 in (0, QL):
                psf = psM.tile([P, 2 * QL], f32, tag="ps", name="ps3")
                ps = psf[:, :QL]
                for kc in range(KC):
                    nc.tensor.matmul(
                        ps, yT_sb[:, kc, tt * P:(tt + 1) * P],
                        wp_sb[:, kc, n0:n0 + QL],
                        start=(kc == 0), stop=(kc == KC - 1),
                    )
                osb = ot.tile([P, QL], f32, tag="osb", name="osb")
                nc.vector.tensor_add(osb[:], ps, bpr_sb[:, n0:n0 + QL])
                nc.sync.dma_start(out[tt * P:(tt + 1) * P, n0:n0 + QL], osb[:])


def _build():
    nc = bacc.Bacc(
        "TRN2", target_bir_lowering=False, debug=False,
        enable_asserts=True, num_devices=N_CORES,
    )
    fp8 = mybir.dt.float8e4
    xt = nc.dram_tensor("xt", [P, KC * HT], bf16, kind="ExternalInput").ap()
    xt8 = nc.dram_tensor("xt8", [P, KC * HT], fp8, kind="ExternalInput").ap()
    wqk8 = nc.dram_tensor("wqk8", [NFT // 2, P, 2 * KC * P], fp8,
                          kind="ExternalInput").ap()
    wv = nc.dram_tensor("wv", [C, C], bf16, kind="ExternalInput").ap()
    wp = nc.dram_tensor("wp", [C, C], bf16, kind="ExternalInput").ap()
    bqk = nc.dram_tensor("bqk", [2 * C], f32, kind="ExternalInput").ap()
    bvr = nc.dram_tensor("bvr", [P, C], bf16, kind="ExternalInput").ap()
    bvr0 = nc.dram_tensor("bvr0", [P, C], bf16, kind="ExternalInput").ap()
    bpr = nc.dram_tensor("bpr", [P, C], bf16, kind="ExternalInput").ap()
    maskF = nc.dram_tensor("maskF", [P, SW], bf16, kind="ExternalInput").ap()
    voner = nc.dram_tensor("voner", [HTP, D], bf16, kind="ExternalInput").ap()
    out = nc.dram_tensor("out", [QL, C], f32, kind="ExternalOutput").ap()
    with tile.TileContext(nc) as tc:
        _emit(tc, xt, xt8, wqk8, wv, wp, bqk, bvr, bvr0, bpr, maskF, voner, out)
    nc.compile()
    return nc


def _get_module():
    if "nc" not in _CACHE:
        _CACHE["nc"] = _build()
    return _CACHE["nc"]


def _band_mask() -> np.ndarray:
    # strip col off+ (q-cs) for chunk c, partition p = local key 128c+p;
    # query qq visible to key kk iff qq <= kk <= qq+64 (halo frame)
    m = np.zeros((P, SW), dtype=np.float32)
    p = np.arange(P)
    for c in range(NKC):
        off, cs, ce = CHUNK_OFF[c], CHUNK_CS[c], CHUNK_CE[c]
        qq = np.arange(cs, ce)[None, :]
        kk = (128 * c + p)[:, None]
        m[:, off:off + ce - cs] = (kk >= qq) & (kk <= qq + W)
    return m.astype(ml_dtypes.bfloat16)


def kernel(x, Wqkv, bqkv, Wproj, bproj):
    x = np.asarray(x, dtype=np.float32)
    Wqkv = np.asarray(Wqkv, dtype=np.float32)
    bqkv = np.asarray(bqkv, dtype=np.float32)
    Wproj = np.asarray(Wproj, dtype=np.float32)
    bproj = np.asarray(bproj, dtype=np.float32)

    bf = ml_dtypes.bfloat16
    f8 = ml_dtypes.float8_e4m3
    # pack Wqk slab-major in the exact SBUF tile layout: [ft, p, kc*128],
    # pre-scaled by QSC so the fp8 values sit in e4m3's sweet spot
    wqk8_np = np.ascontiguousarray(
        (Wqkv[:, :2 * C] * QSC).astype(f8).reshape(KC, P, NFT // 2, 2, P)
        .transpose(2, 1, 3, 0, 4).reshape(NFT // 2, P, 2 * KC * P)
    )
    wv_np = np.ascontiguousarray(Wqkv[:, 2 * C:]).astype(bf)
    wp_np = Wproj.astype(bf)
    bqk_np = np.ascontiguousarray(bqkv[:2 * C] * QSC)
    bvr_np = np.ascontiguousarray(np.broadcast_to(bqkv[2 * C:], (P, C))).astype(bf)
    bvr0_np = bvr_np.copy()
    bvr0_np[:W] = 0.0  # halo-pad tokens of the q==0 cores carry no bias
    bpr_np = np.ascontiguousarray(np.broadcast_to(bproj, (P, C))).astype(bf)
    maskF_np = _band_mask()

    vone_real = np.zeros((HTP, D), dtype=bf)
    vone_real[:HT] = 1.0
    vone_pad = vone_real.copy()
    vone_pad[:W] = 0.0

    in_maps = _prep_in_maps(x, wqk8_np, wv_np, wp_np, bqk_np, bvr_np, bvr0_np,
                            bpr_np, maskF_np, vone_real, vone_pad)

    nc = _get_module()
    _CACHE["last_in_maps"] = in_maps
    res = bass_utils.run_bass_kernel_spmd(nc, in_maps, core_ids=list(range(N_CORES)))

    out = np.empty((B, T, C), dtype=np.float32)
    for c in range(N_CORES):
        b, q = divmod(c, 4)
        out[b, q * QL:(q + 1) * QL] = res.results[c]["out"]
    return out


def _prep_in_maps(x, wqk8_np, wv_np, wp_np, bqk_np, bvr_np, bvr0_np, bpr_np,
                  maskF_np, vone_real, vone_pad):
    bf = ml_dtypes.bfloat16
    f8 = ml_dtypes.float8_e4m3
    in_maps = []
    for c in range(N_CORES):
        b, q = divmod(c, 4)
        lo = q * QL - W
        if lo < 0:
            chunk = np.concatenate(
                [np.zeros((W, C), np.float32), x[b, 0:q * QL + QL]], axis=0
            )
        else:
            chunk = x[b, lo:lo + HT]
        # pack x^T in SBUF tile order: [p, kc*576]
        xtt = chunk.T.reshape(KC, P, HT).transpose(1, 0, 2).reshape(P, KC * HT)
        xt_np = np.ascontiguousarray(xtt.astype(bf))
        xt8_np = np.ascontiguousarray(xtt.astype(f8))
        in_maps.append({
            "xt": xt_np,
            "xt8": xt8_np,
            "wqk8": wqk8_np,
            "wv": wv_np,
            "wp": wp_np,
            "bqk": bqk_np,
            "bvr": bvr_np,
            "bvr0": bvr0_np if q == 0 else bvr_np,
            "bpr": bpr_np,
            "maskF": maskF_np,
            "voner": vone_pad if q == 0 else vone_real,
        })
    return in_maps
